# revision 5
# baseline (speedup 1.0000x reference)
"""Sigmoid-attention kernel for Trainium2, SPMD over 8 NeuronCores.

Reference computation (per batch b, head h):
    q = (x @ Wq_h) * SCALE ; k = x @ Wk_h ; v = x[:, :, h*64:(h+1)*64]
    out_h = sigmoid((q + bias_h) @ k^T) @ v
Sharding: 8 cores = 4 batches x 2 head-groups (4 heads each).
Each core computes its 4 heads independently; no collectives.

Heads are processed in pairs packed into the two 64-partition halves of
the PE array: head A lives on SBUF partitions 0-63, head B on 64-127.
Scores run as two concurrent 64x128 row-tiles; the P@V matmuls run as
two concurrent 128x64 column-tiles writing the two PSUM halves.

The sigmoid over the [n, n] score matrices dominates (ScalarE runs at 1
elem/lane/cycle); every 4th score group is instead evaluated on the
VectorEngine with a Schraudolph exp (bit-trick exp2 via int32 cast) +
reciprocal, balancing ScalarE and VectorE.

All matmuls contract along SBUF partitions, so x arrives pre-transposed
(features on partitions) and the kernel computes q^T/k^T/S^T/out^T;
the host re-transposes the [dk, n] outputs into the reference layout.
"""
import sys

import numpy as np
import ml_dtypes

try:
    import concourse.bass as bass  # noqa: F401
except ImportError:
    sys.path.insert(0, "/opt/trn_rl_repo")
import concourse.tile as tile
from concourse import bacc, mybir
from concourse.bass_utils import run_bass_kernel_spmd

BF16 = mybir.dt.bfloat16
F32 = mybir.dt.float32
I32 = mybir.dt.int32
bf16 = ml_dtypes.bfloat16

B, N, DIM = 4, 2048, 512
HEADS, DK = 8, 64
SCALE = DK ** -0.5
NCORES = 8
HPG = 4            # heads per group (= per core)
NPAIR = HPG // 2   # head pairs per core
GD = HPG * DK      # 256: group feature width
DC = DIM // 128    # 4 d-chunks (contraction tiles for projections)
NIC = N // 512     # 4 i-chunks
NJ = N // 128      # 16 j-tiles

# Schraudolph exp constants (exp(-s) ~= bitcast_f32(int32(B - A*s)))
EXP_A = float(2 ** 23 / np.log(2.0))
EXP_B = float(1064867216)

ACT = mybir.ActivationFunctionType
ALU = mybir.AluOpType


def _build():
    nc = bacc.Bacc("TRN2", target_bir_lowering=False, debug=False)
    xT = nc.declare_dram_parameter("xT", [DIM, N], BF16, isOutput=False)
    wq = nc.declare_dram_parameter("wq", [DIM, GD], BF16, isOutput=False)
    wk = nc.declare_dram_parameter("wk", [DIM, GD], BF16, isOutput=False)
    vv = nc.declare_dram_parameter("v", [N, GD], BF16, isOutput=False)
    bias = nc.declare_dram_parameter("bias", [GD, 1], F32, isOutput=False)
    out = nc.declare_dram_parameter("out", [NPAIR, 128, N], F32, isOutput=True)

    with tile.TileContext(nc) as tc:
        with (
            tc.tile_pool(name="const", bufs=1) as cpool,
            tc.tile_pool(name="qk", bufs=8) as qkpool,
            tc.tile_pool(name="pp", bufs=4) as ppool,
            tc.tile_pool(name="dve", bufs=2) as dvepool,
            tc.tile_pool(name="osb", bufs=2) as opool,
            tc.tile_pool(name="ps_proj", bufs=1, space="PSUM") as pjpool,
            tc.tile_pool(name="ps_s", bufs=2, space="PSUM") as spool,
            tc.tile_pool(name="ps_o", bufs=2, space="PSUM") as oppool,
        ):
            # ---- constants ----
            xt_t = []
            for dc in range(DC):
                t = cpool.tile([128, N], BF16, name=f"xt{dc}")
                nc.sync.dma_start(t[:], xT[dc * 128:(dc + 1) * 128, :])
                xt_t.append(t)
            wq_t, wk_t = [], []
            for dc in range(DC):
                t = cpool.tile([128, GD], BF16, name=f"wqt{dc}")
                nc.sync.dma_start(t[:], wq[dc * 128:(dc + 1) * 128, :])
                wq_t.append(t)
                t = cpool.tile([128, GD], BF16, name=f"wkt{dc}")
                nc.sync.dma_start(t[:], wk[dc * 128:(dc + 1) * 128, :])
                wk_t.append(t)
            # v rearranged so partition p holds v[jc*128+p, :] for each j-chunk jc
            v_t = cpool.tile([128, NJ * GD], BF16, name="vt")
            nc.sync.dma_start(
                v_t[:].rearrange("p (jc e) -> p jc e", jc=NJ),
                vv.rearrange("(jc p) e -> p jc e", p=128),
            )
            bias_t = []
            for p in range(NPAIR):
                t = cpool.tile([128, 1], F32, name=f"bias{p}")
                nc.sync.dma_start(t[:], bias[p * 128:(p + 1) * 128, :])
                bias_t.append(t)

            for p in range(NPAIR):
                ha, hb = 2 * p, 2 * p + 1
                # ---- projections; per-ichunk tiles for fine-grained deps ----
                qbT_t, kT_t = [], []
                for ic in range(NIC):
                    qbT = qkpool.tile([128, 512], BF16, tag="qbT",
                                      name=f"qbT{p}_{ic}")
                    kT = qkpool.tile([128, 512], BF16, tag="kT",
                                     name=f"kT{p}_{ic}")
                    pq = pjpool.tile([128, 512], F32, tag="pq", name=f"pq{p}_{ic}")
                    pk = pjpool.tile([128, 512], F32, tag="pk", name=f"pk{p}_{ic}")
                    cs = slice(ic * 512, (ic + 1) * 512)
                    ws = slice(p * 128, (p + 1) * 128)
                    for dc in range(DC):
                        nc.tensor.matmul(
                            pq[:], wq_t[dc][:, ws], xt_t[dc][:, cs],
                            start=(dc == 0), stop=(dc == DC - 1),
                        )
                    for dc in range(DC):
                        nc.tensor.matmul(
                            pk[:], wk_t[dc][:, ws], xt_t[dc][:, cs],
                            start=(dc == 0), stop=(dc == DC - 1),
                        )
                    # qb = q*SCALE + bias (per-partition), cast to bf16 (DVE)
                    nc.vector.tensor_scalar(qbT[:], pq[:], float(SCALE),
                                            bias_t[p][:, :], ALU.mult, ALU.add)
                    nc.vector.tensor_copy(kT[:], pk[:])
                    qbT_t.append(qbT)
                    kT_t.append(kT)

                # ---- attention ----
                out_sb = opool.tile([128, N], F32, tag="osb", name=f"osb{p}")
                for ic in range(NIC):
                    o_ps = oppool.tile([128, 512], F32, tag="ops", name=f"ops{p}_{ic}")
                    for j in range(NJ):
                        kslc = kT_t[j // 4][:, (j % 4) * 128:(j % 4 + 1) * 128]
                        s_ps = spool.tile([128, 1024], F32, tag="sg",
                                          name=f"s{p}_{ic}_{j}")
                        # head A scores: 64x128 row-tile at rows 0-63
                        nc.tensor.matmul(
                            s_ps[:, 0:512], kslc[0:64, :], qbT_t[ic][0:64, :],
                            start=True, stop=True,
                        )
                        # head B scores: 64x128 row-tile at rows 64-127
                        nc.tensor.matmul(
                            s_ps[:, 512:1024], kslc[64:128, :], qbT_t[ic][64:128, :],
                            start=True, stop=True,
                        )
                        p_sb = ppool.tile([128, 1024], BF16, tag="pg",
                                          name=f"pr{p}_{ic}_{j}")
                        if j % 4 == 3:
                            # VectorE sigmoid: 1/(1 + schraudolph_exp(-s))
                            it = dvepool.tile([128, 1024], I32, tag="sit",
                                              name=f"sit{p}_{ic}_{j}")
                            nc.vector.tensor_scalar(it[:], s_ps[:], -EXP_A, EXP_B,
                                                    ALU.mult, ALU.add)
                            zt = dvepool.tile([128, 1024], F32, tag="szt",
                                              name=f"szt{p}_{ic}_{j}")
                            nc.vector.tensor_scalar_add(zt[:], it[:].bitcast(F32), 1.0)
                            with nc.allow_low_precision(reason="bf16 attn probs"):
                                nc.vector.reciprocal(p_sb[:], zt[:])
                        else:
                            nc.scalar.activation(p_sb[:], s_ps[:], ACT.Sigmoid)
                        # P @ v: two concurrent 128x64 col-tiles into PSUM halves
                        nc.tensor.matmul(
                            o_ps[0:64, :],
                            v_t[:, j * GD + ha * DK: j * GD + (ha + 1) * DK],
                            p_sb[:, 0:512],
                            start=(j == 0), stop=(j == NJ - 1),
                        )
                        nc.tensor.matmul(
                            o_ps[64:128, :],
                            v_t[:, j * GD + hb * DK: j * GD + (hb + 1) * DK],
                            p_sb[:, 512:1024],
                            start=(j == 0), stop=(j == NJ - 1),
                        )
                    nc.vector.tensor_copy(out_sb[:, ic * 512:(ic + 1) * 512], o_ps[:])
                nc.sync.dma_start(out[p], out_sb[:])
    nc.compile()
    return nc


_NC_CACHE = None


def _get_nc():
    global _NC_CACHE
    if _NC_CACHE is None:
        _NC_CACHE = _build()
    return _NC_CACHE


def _make_in_maps(x, Wq, Wk, rb):
    xT_b = [np.ascontiguousarray(x[b].T).astype(bf16) for b in range(B)]
    wq_bf = Wq.astype(bf16)
    wk_bf = Wk.astype(bf16)
    bias_flat = rb.reshape(HEADS * DK, 1)  # [512, 1] head-major

    in_maps = []
    for c in range(NCORES):
        b, g = divmod(c, 2)
        gs = slice(g * GD, (g + 1) * GD)
        in_maps.append({
            "xT": xT_b[b],
            "wq": np.ascontiguousarray(wq_bf[:, gs]),
            "wk": np.ascontiguousarray(wk_bf[:, gs]),
            "v": np.ascontiguousarray(x[b, :, gs]).astype(bf16),
            "bias": np.ascontiguousarray(bias_flat[g * GD:(g + 1) * GD]),
        })
    return in_maps


def _gather(results):
    out_full = np.empty((B, N, DIM), dtype=np.float32)
    for c in range(NCORES):
        b, g = divmod(c, 2)
        oc = results[c]["out"]  # [NPAIR, 128, N]
        for p in range(NPAIR):
            for u in range(2):
                h = 2 * p + u
                col = g * GD + h * DK
                out_full[b, :, col:col + DK] = oc[p, u * 64:(u + 1) * 64, :].T
    return out_full


def kernel(x, Wq, Wk, rel_content_bias):
    x = np.asarray(x, dtype=np.float32)
    Wq = np.asarray(Wq, dtype=np.float32)
    Wk = np.asarray(Wk, dtype=np.float32)
    rb = np.asarray(rel_content_bias, dtype=np.float32)

    nc = _get_nc()
    in_maps = _make_in_maps(x, Wq, Wk, rb)
    res = run_bass_kernel_spmd(nc, in_maps, core_ids=list(range(NCORES)))
    return _gather(res.results)


# revision 6
# speedup vs baseline: 1.6657x; 1.6657x over previous
"""Sigmoid-attention kernel for Trainium2, SPMD over 8 NeuronCores.

Reference computation (per batch b, head h):
    q = (x @ Wq_h) * SCALE ; k = x @ Wk_h ; v = x[:, :, h*64:(h+1)*64]
    out_h = sigmoid((q + bias_h) @ k^T) @ v
Sharding: 8 cores = 4 batches x 2 head-groups (4 heads each).
Each core computes its 4 heads independently; no collectives.

Heads are processed in pairs packed into the two 64-partition halves of
the PE array: head A lives on SBUF partitions 0-63, head B on 64-127.
Scores run as two concurrent 64x128 row-tiles; the P@V matmuls run as
two concurrent 128x64 column-tiles writing the two PSUM halves.

The sigmoid over the [n, n] score matrices dominates (ScalarE runs at 1
elem/lane/cycle); every 4th score group is instead evaluated on the
VectorEngine with a Schraudolph exp (bit-trick exp2 via int32 cast) +
reciprocal, balancing ScalarE and VectorE.

All matmuls contract along SBUF partitions, so x arrives pre-transposed
(features on partitions) and the kernel computes q^T/k^T/S^T/out^T;
the host re-transposes the [dk, n] outputs into the reference layout.
"""
import sys

import numpy as np
import ml_dtypes

try:
    import concourse.bass as bass  # noqa: F401
except ImportError:
    sys.path.insert(0, "/opt/trn_rl_repo")
import concourse.tile as tile
from concourse import bacc, mybir
from concourse.bass_utils import run_bass_kernel_spmd

BF16 = mybir.dt.bfloat16
F32 = mybir.dt.float32
I32 = mybir.dt.int32
bf16 = ml_dtypes.bfloat16

B, N, DIM = 4, 2048, 512
HEADS, DK = 8, 64
SCALE = DK ** -0.5
NCORES = 8
HPG = 4            # heads per group (= per core)
NPAIR = HPG // 2   # head pairs per core
GD = HPG * DK      # 256: group feature width
DC = DIM // 128    # 4 d-chunks (contraction tiles for projections)
NIC = N // 512     # 4 i-chunks
NJ = N // 128      # 16 j-tiles

# Schraudolph exp constants (exp(-s) ~= bitcast_f32(int32(B - A*s)))
EXP_A = float(2 ** 23 / np.log(2.0))
EXP_B = float(1064867216)

ACT = mybir.ActivationFunctionType
ALU = mybir.AluOpType


def _build():
    nc = bacc.Bacc("TRN2", target_bir_lowering=False, debug=False)
    xT = nc.declare_dram_parameter("xT", [DIM, N], BF16, isOutput=False)
    wq = nc.declare_dram_parameter("wq", [DIM, GD], BF16, isOutput=False)
    wk = nc.declare_dram_parameter("wk", [DIM, GD], BF16, isOutput=False)
    vv = nc.declare_dram_parameter("v", [N, GD], BF16, isOutput=False)
    bias = nc.declare_dram_parameter("bias", [GD, 1], F32, isOutput=False)
    out = nc.declare_dram_parameter("out", [NPAIR, 128, N], F32, isOutput=True)

    with tile.TileContext(nc) as tc:
        with (
            tc.tile_pool(name="const", bufs=1) as cpool,
            tc.tile_pool(name="qk", bufs=8) as qkpool,
            tc.tile_pool(name="pp", bufs=4) as ppool,
            tc.tile_pool(name="dve", bufs=2) as dvepool,
            tc.tile_pool(name="osb", bufs=2) as opool,
            tc.tile_pool(name="ps_proj", bufs=1, space="PSUM") as pjpool,
            tc.tile_pool(name="ps_s", bufs=2, space="PSUM") as spool,
            tc.tile_pool(name="ps_o", bufs=2, space="PSUM") as oppool,
        ):
            # ---- constants ----
            xt_t = []
            for dc in range(DC):
                t = cpool.tile([128, N], BF16, name=f"xt{dc}")
                nc.sync.dma_start(t[:], xT[dc * 128:(dc + 1) * 128, :])
                xt_t.append(t)
            wq_t, wk_t = [], []
            for dc in range(DC):
                t = cpool.tile([128, GD], BF16, name=f"wqt{dc}")
                nc.sync.dma_start(t[:], wq[dc * 128:(dc + 1) * 128, :])
                wq_t.append(t)
                t = cpool.tile([128, GD], BF16, name=f"wkt{dc}")
                nc.sync.dma_start(t[:], wk[dc * 128:(dc + 1) * 128, :])
                wk_t.append(t)
            # v rearranged so partition p holds v[jc*128+p, :] for each j-chunk jc
            v_t = cpool.tile([128, NJ * GD], BF16, name="vt")
            nc.sync.dma_start(
                v_t[:].rearrange("p (jc e) -> p jc e", jc=NJ),
                vv.rearrange("(jc p) e -> p jc e", p=128),
            )
            bias_t = []
            for p in range(NPAIR):
                t = cpool.tile([128, 1], F32, name=f"bias{p}")
                nc.sync.dma_start(t[:], bias[p * 128:(p + 1) * 128, :])
                bias_t.append(t)

            for p in range(NPAIR):
                ha, hb = 2 * p, 2 * p + 1
                # ---- projections; per-ichunk tiles for fine-grained deps ----
                qbT_t, kT_t = [], []
                for ic in range(NIC):
                    qbT = qkpool.tile([128, 512], BF16, tag="qbT",
                                      name=f"qbT{p}_{ic}")
                    kT = qkpool.tile([128, 512], BF16, tag="kT",
                                     name=f"kT{p}_{ic}")
                    pq = pjpool.tile([128, 512], F32, tag="pq", name=f"pq{p}_{ic}")
                    pk = pjpool.tile([128, 512], F32, tag="pk", name=f"pk{p}_{ic}")
                    cs = slice(ic * 512, (ic + 1) * 512)
                    ws = slice(p * 128, (p + 1) * 128)
                    for dc in range(DC):
                        nc.tensor.matmul(
                            pq[:], wq_t[dc][:, ws], xt_t[dc][:, cs],
                            start=(dc == 0), stop=(dc == DC - 1),
                        )
                    for dc in range(DC):
                        nc.tensor.matmul(
                            pk[:], wk_t[dc][:, ws], xt_t[dc][:, cs],
                            start=(dc == 0), stop=(dc == DC - 1),
                        )
                    # qb = q*SCALE + bias (per-partition), cast to bf16 (DVE)
                    nc.vector.tensor_scalar(qbT[:], pq[:], float(SCALE),
                                            bias_t[p][:, :], ALU.mult, ALU.add)
                    nc.vector.tensor_copy(kT[:], pk[:])
                    qbT_t.append(qbT)
                    kT_t.append(kT)

                # ---- attention ----
                out_sb = opool.tile([128, N], F32, tag="osb", name=f"osb{p}")
                for ic in range(NIC):
                    o_ps = oppool.tile([128, 512], F32, tag="ops", name=f"ops{p}_{ic}")
                    for j in range(NJ):
                        kslc = kT_t[j // 4][:, (j % 4) * 128:(j % 4 + 1) * 128]
                        s_ps = spool.tile([128, 1024], F32, tag="sg",
                                          name=f"s{p}_{ic}_{j}")
                        # head A scores: 64x128 row-tile at rows 0-63
                        nc.tensor.matmul(
                            s_ps[:, 0:512], kslc[0:64, :], qbT_t[ic][0:64, :],
                            start=True, stop=True,
                        )
                        # head B scores: 64x128 row-tile at rows 64-127
                        nc.tensor.matmul(
                            s_ps[:, 512:1024], kslc[64:128, :], qbT_t[ic][64:128, :],
                            start=True, stop=True,
                        )
                        p_sb = ppool.tile([128, 1024], BF16, tag="pg",
                                          name=f"pr{p}_{ic}_{j}")
                        if j % 16 in (4, 9, 14):
                            # VectorE sigmoid: 1/(1 + schraudolph_exp(-s))
                            it = dvepool.tile([128, 1024], I32, tag="sit",
                                              name=f"sit{p}_{ic}_{j}")
                            nc.vector.tensor_scalar(it[:], s_ps[:], -EXP_A, EXP_B,
                                                    ALU.mult, ALU.add)
                            zt = dvepool.tile([128, 1024], F32, tag="szt",
                                              name=f"szt{p}_{ic}_{j}")
                            nc.vector.tensor_scalar_add(zt[:], it[:].bitcast(F32), 1.0)
                            rt = dvepool.tile([128, 1024], F32, tag="srt",
                                              name=f"srt{p}_{ic}_{j}")
                            nc.vector.reciprocal_approx_fast(rt[:], zt[:])
                            nc.vector.tensor_copy(p_sb[:], rt[:])
                        else:
                            nc.scalar.activation(p_sb[:], s_ps[:], ACT.Sigmoid)
                        # P @ v: two concurrent 128x64 col-tiles into PSUM halves
                        nc.tensor.matmul(
                            o_ps[0:64, :],
                            v_t[:, j * GD + ha * DK: j * GD + (ha + 1) * DK],
                            p_sb[:, 0:512],
                            start=(j == 0), stop=(j == NJ - 1),
                        )
                        nc.tensor.matmul(
                            o_ps[64:128, :],
                            v_t[:, j * GD + hb * DK: j * GD + (hb + 1) * DK],
                            p_sb[:, 512:1024],
                            start=(j == 0), stop=(j == NJ - 1),
                        )
                    nc.vector.tensor_copy(out_sb[:, ic * 512:(ic + 1) * 512], o_ps[:])
                nc.sync.dma_start(out[p], out_sb[:])
    nc.compile()
    return nc


_NC_CACHE = None


def _get_nc():
    global _NC_CACHE
    if _NC_CACHE is None:
        _NC_CACHE = _build()
    return _NC_CACHE


def _make_in_maps(x, Wq, Wk, rb):
    xT_b = [np.ascontiguousarray(x[b].T).astype(bf16) for b in range(B)]
    wq_bf = Wq.astype(bf16)
    wk_bf = Wk.astype(bf16)
    bias_flat = rb.reshape(HEADS * DK, 1)  # [512, 1] head-major

    in_maps = []
    for c in range(NCORES):
        b, g = divmod(c, 2)
        gs = slice(g * GD, (g + 1) * GD)
        in_maps.append({
            "xT": xT_b[b],
            "wq": np.ascontiguousarray(wq_bf[:, gs]),
            "wk": np.ascontiguousarray(wk_bf[:, gs]),
            "v": np.ascontiguousarray(x[b, :, gs]).astype(bf16),
            "bias": np.ascontiguousarray(bias_flat[g * GD:(g + 1) * GD]),
        })
    return in_maps


def _gather(results):
    out_full = np.empty((B, N, DIM), dtype=np.float32)
    for c in range(NCORES):
        b, g = divmod(c, 2)
        oc = results[c]["out"]  # [NPAIR, 128, N]
        for p in range(NPAIR):
            for u in range(2):
                h = 2 * p + u
                col = g * GD + h * DK
                out_full[b, :, col:col + DK] = oc[p, u * 64:(u + 1) * 64, :].T
    return out_full


def kernel(x, Wq, Wk, rel_content_bias):
    x = np.asarray(x, dtype=np.float32)
    Wq = np.asarray(Wq, dtype=np.float32)
    Wk = np.asarray(Wk, dtype=np.float32)
    rb = np.asarray(rel_content_bias, dtype=np.float32)

    nc = _get_nc()
    in_maps = _make_in_maps(x, Wq, Wk, rb)
    res = run_bass_kernel_spmd(nc, in_maps, core_ids=list(range(NCORES)))
    return _gather(res.results)
